# revision 3
# baseline (speedup 1.0000x reference)
"""Bass/Tile TRN2 kernel for nn_BatchGraphAttentionLayer.

Reference computation (per batch b):
    Wh  = h[b] @ W                    # [64, 256]
    s1  = Wh @ a[:256], s2 = Wh @ a[256:]
    e   = leaky_relu(s1[i] + s2[j])   # [64, 64]
    att = softmax over axis i of where(adj[i,j]>0, e, -9e15)
    out = elu(att @ Wh)               # contraction over j

Sharding: data-parallel over batch. 8 cores x 4 batches each.
Each core gets host-pre-transposed hT [16384, 256] (k-major) split into
bf16 hi/lo halves so the projection runs at bf16 PE speed with ~fp32
accuracy (3-term compensated product: hh + hl + lh; the dropped lo*lo
term is O(2^-16) relative).  Same total HBM traffic as fp32.
W is replicated, also split hi/lo.

On-chip layout notes:
  - Projection accumulates Wh in m-major PSUM tiles [128 m, 256 o]
    (m = 4*64 = 256 local rows; two tiles of 128).
  - The 4 batches are processed as 2 "pairs": pair t stacks batches
    (2t, 2t+1) on the 128 partitions.  Attention for a pair is computed
    on a [128, 128] tile whose off-diagonal 64x64 blocks are masked to
    -9e15; after softmax those blocks are exactly 0, so a single
    [128,128] x [128,256] matmul yields both batches' outputs.
  - e^T[j, i] = s2[j] + s1[i] is built with two rank-1 matmuls into
    PSUM (ones (x) s1 and s2 (x) ones).
"""

import os
from contextlib import ExitStack

import ml_dtypes
import numpy as np

import concourse.bass as bass
import concourse.tile as tile
from concourse import bacc, mybir
from concourse.bass_utils import run_bass_kernel_spmd

F32 = mybir.dt.float32
BF16 = mybir.dt.bfloat16

B, N, IN, OUT = 32, 64, 16384, 256
NCORES = 8
BPC = B // NCORES            # batches per core = 4
M = BPC * N                  # local rows = 256
P = 128
NEG = -9e15
ALPHA = 0.2

KSUB = IN // P               # 128 k-subtiles of 128
SLAB_SUB = 16                # k-subtiles per DMA slab
NSLAB = KSUB // SLAB_SUB     # 8 slabs of 2048 k-rows (1 MiB bf16 each)

_NC = None
LAST_EXEC_NS = None
LAST_RESULTS = None


def _build_kernel(ctx: ExitStack, tc: tile.TileContext, out, h_hi, h_lo,
                  w_hi, w_lo, acol, maskmul, maskadd, eye):
    nc = tc.nc

    consts = ctx.enter_context(tc.tile_pool(name="consts", bufs=1))
    hpool = ctx.enter_context(tc.tile_pool(name="hslab", bufs=3))
    wpool = ctx.enter_context(tc.tile_pool(name="wslab", bufs=3))
    whpool = ctx.enter_context(tc.tile_pool(name="wh", bufs=1))
    small = ctx.enter_context(tc.tile_pool(name="small", bufs=2))
    attp = ctx.enter_context(tc.tile_pool(name="att", bufs=2))
    ps_accp = ctx.enter_context(tc.tile_pool(name="psacc", bufs=1, space="PSUM"))
    ps_tmpp = ctx.enter_context(tc.tile_pool(name="pstmp", bufs=1, space="PSUM"))

    # ---- constants ----
    sb_eye = consts.tile([P, P], F32)
    nc.sync.dma_start(sb_eye, eye)
    sb_mm = consts.tile([P, P], F32)
    nc.sync.dma_start(sb_mm, maskmul)
    sb_ma = consts.tile([P, P], F32)
    nc.sync.dma_start(sb_ma, maskadd)
    sb_ones = consts.tile([1, P], F32)
    nc.vector.memset(sb_ones, 1.0)
    # a as 4 column chunks: col j holds a[128j : 128j+128]
    sb_acol = consts.tile([P, 4], F32)
    with nc.allow_non_contiguous_dma(reason="tiny 512-float a vector"):
        for j in range(4):
            nc.sync.dma_start(sb_acol[:, j:j + 1], acol[j * P:(j + 1) * P, :])

    # ---- phase 1: Wh = h @ W, accumulated in PSUM (m-major) ----
    # 3-term compensated bf16 product: hi*hi + hi*lo + lo*hi.
    ps_wh = [ps_accp.tile([P, OUT], F32, tag=f"ps_wh{t}", name=f"ps_wh{t}")
             for t in range(2)]
    for s in range(NSLAB):
        # partition p holds 16 consecutive k-rows -> fully linear DMA.
        # k-subtile c = rows {16p + c}: same k->partition map for h and W,
        # so accumulating over (s, c) contracts every k exactly once.
        ksl = slice(s * SLAB_SUB * P, (s + 1) * SLAB_SUB * P)
        hs_hi = hpool.tile([P, SLAB_SUB, M], BF16, tag="hs_hi")
        nc.sync.dma_start(hs_hi, h_hi[ksl, :].rearrange("(p c) m -> p c m", p=P))
        hs_lo = hpool.tile([P, SLAB_SUB, M], BF16, tag="hs_lo")
        nc.sync.dma_start(hs_lo, h_lo[ksl, :].rearrange("(p c) m -> p c m", p=P))
        ws_hi = wpool.tile([P, SLAB_SUB, OUT], BF16, tag="ws_hi")
        nc.sync.dma_start(ws_hi, w_hi[ksl, :].rearrange("(p c) o -> p c o", p=P))
        ws_lo = wpool.tile([P, SLAB_SUB, OUT], BF16, tag="ws_lo")
        nc.sync.dma_start(ws_lo, w_lo[ksl, :].rearrange("(p c) o -> p c o", p=P))
        first = (s == 0)
        last = (s == NSLAB - 1)
        for c in range(SLAB_SUB):
            for t in range(2):
                msl = slice(t * P, (t + 1) * P)
                st = first and c == 0
                sp = last and c == SLAB_SUB - 1
                nc.tensor.matmul(ps_wh[t], lhsT=hs_hi[:, c, msl],
                                 rhs=ws_hi[:, c, :], start=st, stop=False)
                nc.tensor.matmul(ps_wh[t], lhsT=hs_hi[:, c, msl],
                                 rhs=ws_lo[:, c, :], start=False, stop=False)
                nc.tensor.matmul(ps_wh[t], lhsT=hs_lo[:, c, msl],
                                 rhs=ws_hi[:, c, :], start=False, stop=sp)

    # Wh to SBUF (m-major: [m, o])
    wh_m = [whpool.tile([P, OUT], F32, tag=f"wh_m{t}", name=f"wh_m{t}")
            for t in range(2)]
    for t in range(2):
        nc.vector.tensor_copy(out=wh_m[t], in_=ps_wh[t])

    # ---- phase 2a: WhT (o-major) via PE transposes, then s1/s2 rows ----
    whT = [whpool.tile([P, M], F32, tag=f"whT{c}", name=f"whT{c}")
           for c in range(2)]
    for c in range(2):
        for t in range(2):
            pst = ps_tmpp.tile([P, P], F32, tag="ps_tr")
            nc.tensor.transpose(pst, wh_m[t][:, c * P:(c + 1) * P], sb_eye)
            nc.vector.tensor_copy(out=whT[c][:, t * P:(t + 1) * P], in_=pst)

    s_row = []
    for q in range(2):  # q=0 -> s1 (a[:256]), q=1 -> s2 (a[256:])
        ps_s = ps_tmpp.tile([1, M], F32, tag=f"ps_s{q}")
        for c in range(2):
            nc.tensor.matmul(ps_s, lhsT=sb_acol[:, 2 * q + c:2 * q + c + 1],
                             rhs=whT[c], start=(c == 0), stop=(c == 1))
        sr = small.tile([1, M], F32, tag=f"s_row{q}")
        nc.vector.tensor_copy(out=sr, in_=ps_s)
        s_row.append(sr)

    # ---- phase 2b: per-pair attention ----
    for t in range(2):
        # eT[j~, i~] = s2[j~] + s1[i~] via two rank-1 matmuls
        ps_e = ps_tmpp.tile([P, P], F32, tag="ps_e")
        nc.tensor.matmul(ps_e, lhsT=s_row[1][0:1, t * P:(t + 1) * P],
                         rhs=sb_ones, start=True, stop=False)
        nc.tensor.matmul(ps_e, lhsT=sb_ones,
                         rhs=s_row[0][0:1, t * P:(t + 1) * P],
                         start=False, stop=True)
        # leaky relu: max(x, 0.2x), then mask: lk*maskmul + maskadd
        lk = attp.tile([P, P], F32, tag="lk")
        nc.vector.tensor_scalar_mul(lk, ps_e, ALPHA)
        nc.vector.tensor_tensor(lk, lk, ps_e, mybir.AluOpType.max)
        nc.vector.tensor_tensor(lk, lk, sb_mm, mybir.AluOpType.mult)
        nc.vector.tensor_tensor(lk, lk, sb_ma, mybir.AluOpType.add)
        # softmax along free dim
        nmax = small.tile([P, 1], F32, tag="nmax")
        nc.vector.tensor_reduce(nmax, lk, axis=mybir.AxisListType.X,
                                op=mybir.AluOpType.max, negate=True)
        pexp = attp.tile([P, P], F32, tag="pexp")
        rsum = small.tile([P, 1], F32, tag="rsum")
        nc.scalar.activation(pexp, lk, mybir.ActivationFunctionType.Exp,
                             bias=nmax, scale=1.0, accum_out=rsum)
        rinv = small.tile([P, 1], F32, tag="rinv")
        nc.vector.reciprocal(rinv, rsum)
        att = attp.tile([P, P], F32, tag="att")
        nc.vector.tensor_scalar_mul(att, pexp, rinv)
        # out[i~, o] = sum_j~ att[j~, i~] * Wh[j~, o]  (off-diag blocks are 0)
        ps_o = ps_tmpp.tile([P, OUT], F32, tag="ps_o")
        nc.tensor.matmul(ps_o, lhsT=att, rhs=wh_m[t], start=True, stop=True)
        # elu(x) = max(x,0)-1 + exp(min(x,0))
        m0 = attp.tile([P, OUT], F32, tag="m0")
        nc.vector.tensor_scalar_min(m0, ps_o, 0.0)
        ex = attp.tile([P, OUT], F32, tag="ex")
        nc.scalar.activation(ex, m0, mybir.ActivationFunctionType.Exp)
        rm1 = attp.tile([P, OUT], F32, tag="rm1")
        nc.vector.tensor_scalar(rm1, ps_o, 0.0, -1.0,
                                mybir.AluOpType.max, mybir.AluOpType.add)
        ot = attp.tile([P, OUT], F32, tag="ot")
        nc.vector.tensor_tensor(ot, ex, rm1, mybir.AluOpType.add)
        nc.sync.dma_start(out[t * P:(t + 1) * P, :], ot)


def _get_nc():
    global _NC
    if _NC is not None:
        return _NC
    nc = bacc.Bacc("TRN2", target_bir_lowering=False, debug=False,
                   num_devices=NCORES)
    h_hi = nc.dram_tensor("h_hi", [IN, M], BF16, kind="ExternalInput").ap()
    h_lo = nc.dram_tensor("h_lo", [IN, M], BF16, kind="ExternalInput").ap()
    w_hi = nc.dram_tensor("w_hi", [IN, OUT], BF16, kind="ExternalInput").ap()
    w_lo = nc.dram_tensor("w_lo", [IN, OUT], BF16, kind="ExternalInput").ap()
    acol = nc.dram_tensor("acol", [2 * OUT, 1], F32, kind="ExternalInput").ap()
    maskmul = nc.dram_tensor("maskmul", [P, P], F32, kind="ExternalInput").ap()
    maskadd = nc.dram_tensor("maskadd", [P, P], F32, kind="ExternalInput").ap()
    eye = nc.dram_tensor("eye", [P, P], F32, kind="ExternalInput").ap()
    out = nc.dram_tensor("out", [M, OUT], F32, kind="ExternalOutput").ap()
    with tile.TileContext(nc) as tc:
        with ExitStack() as ctx:
            _build_kernel(ctx, tc, out, h_hi, h_lo, w_hi, w_lo, acol,
                          maskmul, maskadd, eye)
    nc.compile()
    _NC = nc
    return nc


def _masks(adj: np.ndarray):
    adjb = (np.asarray(adj) > 0)                 # [i, j]
    mm = np.zeros((P, P), np.float32)
    mm[:N, :N] = adjb.T.astype(np.float32)       # [j, i]
    mm[N:, N:] = adjb.T.astype(np.float32)
    ma = np.where(mm > 0, np.float32(0.0), np.float32(NEG)).astype(np.float32)
    return mm, ma


def _split_hi_lo(x: np.ndarray):
    hi = x.astype(ml_dtypes.bfloat16)
    lo = (x - hi.astype(np.float32)).astype(ml_dtypes.bfloat16)
    return hi, lo


def kernel(h: np.ndarray, adj: np.ndarray, W: np.ndarray, a: np.ndarray
           ) -> np.ndarray:
    global LAST_EXEC_NS, LAST_RESULTS
    h = np.asarray(h, dtype=np.float32)
    W = np.asarray(W, dtype=np.float32)
    a = np.ascontiguousarray(np.asarray(a, dtype=np.float32)).reshape(2 * OUT, 1)
    assert h.shape == (B, N, IN) and W.shape == (IN, OUT)

    nc = _get_nc()
    mm, ma = _masks(adj)
    eye = np.eye(P, dtype=np.float32)
    w_hi, w_lo = _split_hi_lo(W)

    in_maps = []
    for c in range(NCORES):
        hT = h[c * BPC:(c + 1) * BPC].reshape(M, IN).T
        h_hi, h_lo = _split_hi_lo(np.ascontiguousarray(hT))
        in_maps.append({"h_hi": h_hi, "h_lo": h_lo, "w_hi": w_hi,
                        "w_lo": w_lo, "acol": a, "maskmul": mm,
                        "maskadd": ma, "eye": eye})

    trace = os.environ.get("GAT_TRACE", "0") == "1"
    res = run_bass_kernel_spmd(nc, in_maps, list(range(NCORES)), trace=trace)
    LAST_EXEC_NS = res.exec_time_ns
    LAST_RESULTS = res

    out = np.empty((B, N, OUT), np.float32)
    for c in range(NCORES):
        out[c * BPC:(c + 1) * BPC] = res.results[c]["out"].reshape(BPC, N, OUT)
    return out
